# revision 3
# baseline (speedup 1.0000x reference)
"""MinkowskiGlobalPooling (average=True) segment-mean kernel for 8 trn2 cores.

Full inputs in, full output out. Internally:
  - rows are sharded across 8 cores (500k rows each),
  - each core computes partial per-batch sums and counts via one-hot matmuls
    (mask[p,b] = (batch_idx[p]==b), PSUM accumulates [B, C+1] where the last
    column carries counts thanks to a host-appended ones-column on feats),
  - host sums the 8 partials and divides.
"""

import numpy as np


def _ensure_import_path():
    try:
        import concourse.bass  # noqa: F401
    except ImportError:
        import sys

        for p in ("/opt/trn_rl_repo", "/root/.axon_site/_ro/trn_rl_repo"):
            if p not in sys.path:
                sys.path.insert(0, p)


N_CORES = 8
B = 32  # batches
C = 64  # channels
CP1 = C + 1  # channels + ones column
N_TOTAL = 4_000_000
N_CORE = N_TOTAL // N_CORES  # 500_000
P = 125  # SBUF partitions used (N_CORE = P * R exactly)
R = N_CORE // P  # 4000 rows per partition
T = 100  # rows per chunk (per partition)
NCHUNK = R // T  # 40


def build_program(p=P, r=R, t=T, col_groups=1):
    """Build the per-core Bass program. All cores run the identical program.

    col_groups: number of PE column groups (1 or 4). With 4, consecutive
    row-slots go to different 32-col strips of the PE array and accumulate
    into different psum partition strips; host sums the strips.
    """
    _ensure_import_path()
    import concourse.mybir as mybir
    from concourse import bacc
    from concourse.tile import TileContext

    f32 = mybir.dt.float32
    nchunk = r // t
    assert r % t == 0

    nc = bacc.Bacc()
    feats = nc.dram_tensor("feats", [p * r, CP1], f32, kind="ExternalInput")
    idx = nc.dram_tensor("idx", [p * r], f32, kind="ExternalInput")
    iota = nc.dram_tensor("iota", [p, t * B], f32, kind="ExternalInput")
    out = nc.dram_tensor("out", [col_groups * B, CP1], f32, kind="ExternalOutput")

    feats_r = feats[:, :].rearrange("(p r) c -> p (r c)", p=p)  # [p, r*65]
    idx_r = idx[:].rearrange("(p r) -> p r", p=p)  # [p, r]

    with TileContext(nc) as tc:
        with (
            tc.tile_pool(name="const", bufs=1) as cpool,
            tc.tile_pool(name="feats", bufs=3) as fpool,
            tc.tile_pool(name="mask", bufs=3) as mpool,
            tc.tile_pool(name="psum", bufs=1, space="PSUM") as ppool,
            tc.tile_pool(name="outp", bufs=1) as opool,
        ):
            idx_sb = cpool.tile([p, r], f32)
            nc.sync.dma_start(out=idx_sb[:], in_=idx_r)
            iota_sb = cpool.tile([p, t * B], f32)
            nc.sync.dma_start(out=iota_sb[:], in_=iota[:, :])

            psum = ppool.tile([col_groups * B, CP1], f32)
            n_mm = nchunk * t
            k = 0
            for j in range(nchunk):
                ft = fpool.tile([p, t * CP1], f32)
                nc.sync.dma_start(
                    out=ft[:], in_=feats_r[:, j * t * CP1 : (j + 1) * t * CP1]
                )
                mk = mpool.tile([p, t * B], f32)
                nc.vector.tensor_tensor(
                    out=mk[:].rearrange("p (t b) -> p t b", b=B),
                    in0=idx_sb[:, j * t : (j + 1) * t]
                    .unsqueeze(2)
                    .to_broadcast([p, t, B]),
                    in1=iota_sb[:].rearrange("p (t b) -> p t b", b=B),
                    op=mybir.AluOpType.is_equal,
                )
                for tt in range(t):
                    g = tt % col_groups
                    nc.tensor.matmul(
                        psum[g * B : (g + 1) * B, :],
                        lhsT=mk[:, tt * B : (tt + 1) * B],
                        rhs=ft[:, tt * CP1 : (tt + 1) * CP1],
                        start=(k < col_groups),
                        stop=(k >= n_mm - col_groups),
                        tile_position=(0, g * B) if col_groups > 1 else None,
                        skip_group_check=(col_groups > 1),
                    )
                    k += 1
            out_sb = opool.tile([col_groups * B, CP1], f32)
            nc.vector.tensor_copy(out=out_sb[:], in_=psum[:])
            nc.sync.dma_start(out=out[:, :], in_=out_sb[:])
    nc.finalize()
    return nc


def host_prep(feats, batch_idx):
    """Build per-core input maps from full inputs."""
    feats = np.ascontiguousarray(np.asarray(feats, dtype=np.float32))
    bi = np.asarray(batch_idx)
    n, c = feats.shape
    assert n == N_TOTAL and c == C, (n, c)

    feats65 = np.empty((n, CP1), dtype=np.float32)
    feats65[:, :C] = feats
    feats65[:, C] = 1.0
    idxf = bi.astype(np.float32)
    iota_rep = np.tile(np.arange(B, dtype=np.float32), (P, T))  # [P, T*B]

    in_maps = []
    for m in range(N_CORES):
        sl = slice(m * N_CORE, (m + 1) * N_CORE)
        in_maps.append(
            {"feats": feats65[sl], "idx": idxf[sl], "iota": iota_rep}
        )
    return in_maps


_CACHED_NC = None


def get_program():
    global _CACHED_NC
    if _CACHED_NC is None:
        _CACHED_NC = build_program()
    return _CACHED_NC


def run_on_cores(in_maps, trace=False):
    _ensure_import_path()
    from concourse.bass_utils import run_bass_kernel_spmd

    nc = get_program()
    res = run_bass_kernel_spmd(nc, in_maps, list(range(N_CORES)), trace=trace)
    return res


def finalize(per_core_outs):
    acc = np.zeros((B, CP1), dtype=np.float64)
    for o in per_core_outs:
        o = np.asarray(o, dtype=np.float64)
        # col_groups > 1 leaves strips stacked on the partition axis
        acc += o.reshape(-1, B, CP1).sum(axis=0)
    sums = acc[:, :C]
    counts = acc[:, C]
    pooled = sums / np.maximum(counts, 1.0)[:, None]
    return pooled.astype(np.float32)


def kernel(feats, batch_idx, num_batches):
    assert int(num_batches) == B
    in_maps = host_prep(feats, batch_idx)
    res = run_on_cores(in_maps)
    return finalize([r["out"] for r in res.results])


# revision 7
# speedup vs baseline: 1.1920x; 1.1920x over previous
"""MinkowskiGlobalPooling (average=True) segment-mean kernel for 8 trn2 cores.

Full inputs in, full output out. Internally:
  - rows are sharded across 8 cores (500k rows each),
  - each core computes partial per-batch sums and counts via one-hot matmuls
    (mask[p,b] = (batch_idx[p]==b), PSUM accumulates [B, C+1] where the last
    column carries counts thanks to a host-appended ones-column on feats),
  - host sums the 8 partials and divides.
"""

import numpy as np


def _ensure_import_path():
    try:
        import concourse.bass  # noqa: F401
    except ImportError:
        import sys

        for p in ("/opt/trn_rl_repo", "/root/.axon_site/_ro/trn_rl_repo"):
            if p not in sys.path:
                sys.path.insert(0, p)


N_CORES = 8
B = 32  # batches
C = 64  # channels
CP1 = C + 1  # channels + ones column
N_TOTAL = 4_000_000
N_CORE = N_TOTAL // N_CORES  # 500_000
P = 125  # SBUF partitions used (N_CORE = P * R exactly)
R = N_CORE // P  # 4000 rows per partition
T = 50  # rows per chunk (per partition)
NCHUNK = R // T  # 80


def build_program(p=P, r=R, t=T, col_groups=1):
    """Build the per-core Bass program. All cores run the identical program.

    col_groups: number of PE column groups (1 or 4). With 4, consecutive
    row-slots go to different 32-col strips of the PE array and accumulate
    into different psum partition strips; host sums the strips.
    """
    _ensure_import_path()
    import concourse.mybir as mybir
    from concourse import bacc
    from concourse.tile import TileContext

    f32 = mybir.dt.float32
    nchunk = r // t
    assert r % t == 0

    nc = bacc.Bacc()
    feats = nc.dram_tensor("feats", [p * r, CP1], f32, kind="ExternalInput")
    idx = nc.dram_tensor("idx", [p * r], f32, kind="ExternalInput")
    iota = nc.dram_tensor("iota", [p, t * B], f32, kind="ExternalInput")
    out = nc.dram_tensor("out", [col_groups * B, CP1], f32, kind="ExternalOutput")

    feats_r = feats[:, :].rearrange("(p r) c -> p (r c)", p=p)  # [p, r*65]
    idx_r = idx[:].rearrange("(p r) -> p r", p=p)  # [p, r]

    with TileContext(nc) as tc:
        with (
            tc.tile_pool(name="const", bufs=1) as cpool,
            tc.tile_pool(name="feats", bufs=6) as fpool,
            tc.tile_pool(name="mask", bufs=4) as mpool,
            tc.tile_pool(name="psum", bufs=1, space="PSUM") as ppool,
            tc.tile_pool(name="outp", bufs=1) as opool,
        ):
            idx_sb = cpool.tile([p, r], f32)
            nc.scalar.dma_start(out=idx_sb[:], in_=idx_r)
            iota_sb = cpool.tile([p, t * B], f32)
            nc.gpsimd.dma_start(out=iota_sb[:], in_=iota[:, :])

            psum = ppool.tile([col_groups * B, CP1], f32)
            n_mm = nchunk * t
            k = 0
            dma_engines = [nc.sync, nc.scalar, nc.gpsimd]
            for j in range(nchunk):
                ft = fpool.tile([p, t * CP1], f32)
                dma_engines[j % len(dma_engines)].dma_start(
                    out=ft[:], in_=feats_r[:, j * t * CP1 : (j + 1) * t * CP1]
                )
                mk = mpool.tile([p, t * B], f32)
                nc.vector.tensor_tensor(
                    out=mk[:].rearrange("p (t b) -> p t b", b=B),
                    in0=idx_sb[:, j * t : (j + 1) * t]
                    .unsqueeze(2)
                    .to_broadcast([p, t, B]),
                    in1=iota_sb[:].rearrange("p (t b) -> p t b", b=B),
                    op=mybir.AluOpType.is_equal,
                )
                for tt in range(t):
                    g = tt % col_groups
                    nc.tensor.matmul(
                        psum[g * B : (g + 1) * B, :],
                        lhsT=mk[:, tt * B : (tt + 1) * B],
                        rhs=ft[:, tt * CP1 : (tt + 1) * CP1],
                        start=(k < col_groups),
                        stop=(k >= n_mm - col_groups),
                        tile_position=(0, g * B) if col_groups > 1 else None,
                        skip_group_check=(col_groups > 1),
                    )
                    k += 1
            out_sb = opool.tile([col_groups * B, CP1], f32)
            nc.vector.tensor_copy(out=out_sb[:], in_=psum[:])
            nc.sync.dma_start(out=out[:, :], in_=out_sb[:])
    nc.finalize()
    return nc


def host_prep(feats, batch_idx):
    """Build per-core input maps from full inputs."""
    feats = np.ascontiguousarray(np.asarray(feats, dtype=np.float32))
    bi = np.asarray(batch_idx)
    n, c = feats.shape
    assert n == N_TOTAL and c == C, (n, c)

    feats65 = np.empty((n, CP1), dtype=np.float32)
    feats65[:, :C] = feats
    feats65[:, C] = 1.0
    idxf = bi.astype(np.float32)
    iota_rep = np.tile(np.arange(B, dtype=np.float32), (P, T))  # [P, T*B]

    in_maps = []
    for m in range(N_CORES):
        sl = slice(m * N_CORE, (m + 1) * N_CORE)
        in_maps.append(
            {"feats": feats65[sl], "idx": idxf[sl], "iota": iota_rep}
        )
    return in_maps


_CACHED_NC = None


def get_program():
    global _CACHED_NC
    if _CACHED_NC is None:
        _CACHED_NC = build_program()
    return _CACHED_NC


def run_on_cores(in_maps, trace=False):
    _ensure_import_path()
    from concourse.bass_utils import run_bass_kernel_spmd

    nc = get_program()
    res = run_bass_kernel_spmd(nc, in_maps, list(range(N_CORES)), trace=trace)
    return res


def finalize(per_core_outs):
    acc = np.zeros((B, CP1), dtype=np.float64)
    for o in per_core_outs:
        o = np.asarray(o, dtype=np.float64)
        # col_groups > 1 leaves strips stacked on the partition axis
        acc += o.reshape(-1, B, CP1).sum(axis=0)
    sums = acc[:, :C]
    counts = acc[:, C]
    pooled = sums / np.maximum(counts, 1.0)[:, None]
    return pooled.astype(np.float32)


def kernel(feats, batch_idx, num_batches):
    assert int(num_batches) == B
    in_maps = host_prep(feats, batch_idx)
    res = run_on_cores(in_maps)
    return finalize([r["out"] for r in res.results])


# revision 10
# speedup vs baseline: 1.6771x; 1.4070x over previous
"""MinkowskiGlobalPooling (average=True) segment-mean kernel for 8 trn2 cores.

Full inputs in, full output out. Internally:
  - rows are sharded across 8 cores (500k rows each),
  - each core computes partial per-batch sums and counts via one-hot matmuls
    (mask[p,b] = (batch_idx[p]==b), PSUM accumulates [B, C+1] where the last
    column carries counts thanks to a host-appended ones-column on feats),
  - host sums the 8 partials and divides.
"""

import numpy as np


def _ensure_import_path():
    try:
        import concourse.bass  # noqa: F401
    except ImportError:
        import sys

        for p in ("/opt/trn_rl_repo", "/root/.axon_site/_ro/trn_rl_repo"):
            if p not in sys.path:
                sys.path.insert(0, p)


N_CORES = 8
B = 32  # batches
C = 64  # channels
CP1 = C + 1  # channels + ones column
N_TOTAL = 4_000_000
N_CORE = N_TOTAL // N_CORES  # 500_000
P = 125  # SBUF partitions used (N_CORE = P * R exactly)
R = N_CORE // P  # 4000 rows per partition
T = 100  # rows per chunk (per partition)
NCHUNK = R // T  # 40


def build_program(p=P, r=R, t=T, col_groups=1):
    """Build the per-core Bass program. All cores run the identical program.

    col_groups: number of PE column groups (1 or 4). With 4, consecutive
    row-slots go to different 32-col strips of the PE array and accumulate
    into different psum partition strips; host sums the strips.
    """
    _ensure_import_path()
    import concourse.mybir as mybir
    from concourse import bacc
    from concourse.tile import TileContext

    f32 = mybir.dt.float32
    nchunk = r // t
    assert r % t == 0

    nc = bacc.Bacc()
    feats = nc.dram_tensor("feats", [p * r, CP1], f32, kind="ExternalInput")
    idx = nc.dram_tensor("idx", [p * r], f32, kind="ExternalInput")
    iota = nc.dram_tensor("iota", [p, t * B], f32, kind="ExternalInput")
    out = nc.dram_tensor("out", [col_groups * B, CP1], f32, kind="ExternalOutput")

    feats_r = feats[:, :].rearrange("(p r) c -> p (r c)", p=p)  # [p, r*65]
    idx_r = idx[:].rearrange("(p r) -> p r", p=p)  # [p, r]

    with TileContext(nc) as tc:
        with (
            tc.tile_pool(name="const", bufs=1) as cpool,
            tc.tile_pool(name="feats", bufs=4) as fpool,
            tc.tile_pool(name="mask", bufs=3) as mpool,
            tc.tile_pool(name="psum", bufs=1, space="PSUM") as ppool,
            tc.tile_pool(name="outp", bufs=1) as opool,
        ):
            idx_sb = cpool.tile([p, r], f32)
            nc.scalar.dma_start(out=idx_sb[:], in_=idx_r)
            iota_sb = cpool.tile([p, t * B], f32)
            nc.gpsimd.dma_start(out=iota_sb[:], in_=iota[:, :])

            psum = ppool.tile([col_groups * B, CP1], f32)
            n_mm = nchunk * t
            k = 0
            for j in range(nchunk):
                ft = fpool.tile([p, t * CP1], f32)
                nc.gpsimd.dma_start(
                    out=ft[:], in_=feats_r[:, j * t * CP1 : (j + 1) * t * CP1]
                )
                mk = mpool.tile([p, t * B], f32)
                nc.vector.tensor_tensor(
                    out=mk[:].rearrange("p (t b) -> p t b", b=B),
                    in0=idx_sb[:, j * t : (j + 1) * t]
                    .unsqueeze(2)
                    .to_broadcast([p, t, B]),
                    in1=iota_sb[:].rearrange("p (t b) -> p t b", b=B),
                    op=mybir.AluOpType.is_equal,
                )
                for tt in range(t):
                    g = tt % col_groups
                    nc.tensor.matmul(
                        psum[g * B : (g + 1) * B, :],
                        lhsT=mk[:, tt * B : (tt + 1) * B],
                        rhs=ft[:, tt * CP1 : (tt + 1) * CP1],
                        start=(k < col_groups),
                        stop=(k >= n_mm - col_groups),
                        tile_position=(0, g * B) if col_groups > 1 else None,
                        skip_group_check=(col_groups > 1),
                    )
                    k += 1
            out_sb = opool.tile([col_groups * B, CP1], f32)
            nc.vector.tensor_copy(out=out_sb[:], in_=psum[:])
            nc.sync.dma_start(out=out[:, :], in_=out_sb[:])
    nc.finalize()
    return nc


def host_prep(feats, batch_idx):
    """Build per-core input maps from full inputs."""
    feats = np.ascontiguousarray(np.asarray(feats, dtype=np.float32))
    bi = np.asarray(batch_idx)
    n, c = feats.shape
    assert n == N_TOTAL and c == C, (n, c)

    feats65 = np.empty((n, CP1), dtype=np.float32)
    feats65[:, :C] = feats
    feats65[:, C] = 1.0
    idxf = bi.astype(np.float32)
    iota_rep = np.tile(np.arange(B, dtype=np.float32), (P, T))  # [P, T*B]

    in_maps = []
    for m in range(N_CORES):
        sl = slice(m * N_CORE, (m + 1) * N_CORE)
        in_maps.append(
            {"feats": feats65[sl], "idx": idxf[sl], "iota": iota_rep}
        )
    return in_maps


_CACHED_NC = None


def get_program():
    global _CACHED_NC
    if _CACHED_NC is None:
        _CACHED_NC = build_program()
    return _CACHED_NC


def run_on_cores(in_maps, trace=False):
    _ensure_import_path()
    from concourse.bass_utils import run_bass_kernel_spmd

    nc = get_program()
    res = run_bass_kernel_spmd(nc, in_maps, list(range(N_CORES)), trace=trace)
    return res


def finalize(per_core_outs):
    acc = np.zeros((B, CP1), dtype=np.float64)
    for o in per_core_outs:
        o = np.asarray(o, dtype=np.float64)
        # col_groups > 1 leaves strips stacked on the partition axis
        acc += o.reshape(-1, B, CP1).sum(axis=0)
    sums = acc[:, :C]
    counts = acc[:, C]
    pooled = sums / np.maximum(counts, 1.0)[:, None]
    return pooled.astype(np.float32)


def kernel(feats, batch_idx, num_batches):
    assert int(num_batches) == B
    in_maps = host_prep(feats, batch_idx)
    res = run_on_cores(in_maps)
    return finalize([r["out"] for r in res.results])
